# revision 2
# baseline (speedup 1.0000x reference)
"""GPTQ int4 linear kernel for Trainium2, 8-way sharded over out_features (v3).

y = x @ W_dq^T + bias; W_dq group-dequantized from int4 nibbles (two per
byte, only the low byte of each int32 of qweight_packed is meaningful).

v3 design (vs v2): flipped matmul orientation + fp8 subnormal decode.
  - Weights stream as the same host-pre-transposed int16 lane tiles
    qt[p, j, o] (byte pair = 4 nibbles for k = 4l..4l+3, l = 128j+p).
  - DVE decode, 2 passes per tile:  A8 = v & 0x0F0F, B8 = (v>>4) & 0x0F0F.
    Each int16 lane then holds TWO fp8e4m3 atoms whose bit patterns are
    raw nibbles: e4m3 pattern n (0..15) = n * 2^-9 EXACTLY (subnormals for
    n<8; verified exact on HW). A8 = (n@k=4l, n@k=4l+2), B8 = (4l+1, 4l+3).
  - Stage-1 matmuls run in fp8 DoubleRow perf mode (2 contraction rows per
    element pair, 0.5 cyc/col): stationary = block-diag-masked x as e4m3
    PAIRS, split-layout [128, 2, 128(g',b)] (walrus s3_lw_dual_fp8 rejects
    interleaved lhsT; interleaved RHS is fine, so the A8/B8 tiles are used
    directly via stride-2 fp8 views). x is split x = xh + xl (both e4m3,
    error feedback) -> 2 stationary sets; residual ~1.1e-3 relative.
  - PSUM layout [(g',b), o-third]: partitions = 4 group-blocks x 32 batch,
    free = 512 output cols (1 bank, bank-aligned). Accumulates A8/B8 x
    (xh, xl) = 4 matmuls per (j, third).
  - Evict+scale: ACT copies psum -> bf16 (some thirds go DVE-direct), DVE
    multiplies by scb (host-expanded s*2^9 in bf16, streamed per iter) and
    accumulates over j in bf16 (subnormal decode leaves no giant offsets,
    so bf16 is safe here).
  - Stage 2: one matmul per 512-col chunk with stationary = tiled identity
    IND[p, b] = (p%32 == b) contracts the (g',b) partitions -> psum2
    [32 b, o] f32; correction matmuls (zp vs true-x group sums, -8s vs
    (xh+xl) group sums, bias) accumulate into the same psum2 in fp16.
  - ACT evicts psum2 -> DMA out yT [32, 1408] f32.
Host does byte-permutation + small-tensor math only (scales/sums/x-split;
no weight-volume arithmetic).
"""

import sys

for _p in ("/opt/trn_rl_repo",):
    if _p not in sys.path:
        sys.path.insert(0, _p)

import numpy as np

import concourse.bacc as bacc
import concourse.bass as bass
import concourse.mybir as mybir
from concourse import tile

# Problem constants (hardcoded per contract)
OUT_F = 11008
IN_F = 4096
GROUP = 128
N_GROUPS = IN_F // GROUP  # 32
BATCH = 32
N_CORES = 8
SHARD = OUT_F // N_CORES      # 1376
SHARD_P = 1408                # padded to a multiple of 128
NJ = 8                        # int16 lane tiles (1024 lanes / 128)
NZ = 2 * N_GROUPS + 1         # correction rows: zp, -8s, bias
NT = 3                        # o-thirds (512, 512, 384)
TW = (512, 512, 384)
TO = (0, 512, 1024)

F32 = mybir.dt.float32
F16 = mybir.dt.float16
BF16 = mybir.dt.bfloat16
F8 = mybir.dt.float8e4
I16 = mybir.dt.int16

AUXW = SHARD_P + BATCH        # z cols | xs cols (fp16 rows 0..NZ)


def build_nc(repeat=1, unroll=16, dve_evict=0, pool_adds=8, double_row=True,
             ps_bufs=5, qt_bufs=16, pl_bufs=2, ev_bufs=6, scb_bufs=2,
             t_inner=True, s2_per_j=True, s2_from_j=0, s2_lag=-1,
             aux_ring="gpsimd",
             debug_skip=()):
    """Single-core program (SPMD across 8 cores, data differs).

    dve_evict: how many of the 24 (j,third) eviction slots skip the ACT
      copy and run the scale multiply directly on PSUM from DVE (1x mode)
      to balance ACT vs DVE.
    pool_adds: how many of the 21 j-accumulation adds run on the (idle)
      Pool engine instead of DVE.
    t_inner: order stage-1 matmuls (xp, pl) outer / t inner so consecutive
      matmuls share a stationary (Ldweights dedup probe).
    double_row: fp8 DoubleRow perf mode for stage-1 (fallback: plain fp8).
    """
    nc = bacc.Bacc("TRN2", target_bir_lowering=False, debug=False)

    qt_d = nc.dram_tensor("qt", [128, NJ * SHARD_P], I16, kind="ExternalInput")
    # compact stationary x: [p, j, pl, xp, i, b-pairs] int16 (fp8 pairs)
    mqc_d = nc.dram_tensor("mqc", [128, NJ * 8 * 16], I16, kind="ExternalInput")
    # expanded scales (s * 2^9, bf16): scb[p, j, o] = s2[o, 4j + p//32]
    scb_d = nc.dram_tensor("scb", [128, NJ * SHARD_P], F16, kind="ExternalInput")
    # aux fp16: rows 0..NZ: z (cols 0:1408) | xs (cols 1408:1440)
    aux_d = nc.dram_tensor("aux", [NZ, AUXW], F16, kind="ExternalInput")
    # stage-2 stationary: tiled identity, constant
    ind_d = nc.dram_tensor("ind", [128, BATCH], F16, kind="ExternalInput")
    yt_d = nc.dram_tensor("yT", [BATCH, SHARD_P], F32, kind="ExternalOutput")

    with tile.TileContext(nc) as tc:
        with (
            tc.tile_pool(name="xc", bufs=4) as xc,
            tc.tile_pool(name="qt", bufs=qt_bufs) as qtp,
            tc.tile_pool(name="pl", bufs=pl_bufs) as plp,
            tc.tile_pool(name="ev", bufs=ev_bufs) as evp,
            tc.tile_pool(name="scb", bufs=scb_bufs) as scbp,
            tc.tile_pool(name="ps", bufs=ps_bufs, space="PSUM") as psp,
            tc.tile_pool(name="ps2", bufs=1, space="PSUM") as ps2p,
        ):
            # constants / fixed buffers (outside the hw loop)
            ind = xc.tile([128, BATCH], F16, tag="ind", bufs=1)
            nc.sync.dma_start(ind[:], ind_d[:])
            # masked stationary mega-tile [p, j, pl, xp, i, col] fp8,
            # zeroed once; diag blocks rewritten per body
            mqf = xc.tile([128, NJ, 2, 2, 2, 128], F8, tag="mqf", bufs=1)
            nc.vector.memset(mqf[:].bitcast(I16), 0)
            # ACT function table warm
            actwarm = xc.tile([128, 1], F32, tag="actwarm", bufs=1)
            nc.vector.memset(actwarm[:], 0.0)
            nc.scalar.activation(
                actwarm[:], actwarm[:], mybir.ActivationFunctionType.Copy
            )

            def body(it):
                mqc = xc.tile([128, NJ, 8, 16], I16, tag="mqc", name=f"mqc{it}")
                aux = xc.tile([NZ, AUXW], F16, tag="aux", name=f"aux{it}")
                scb = scbp.tile([128, NJ, SHARD_P], F16, tag="scb",
                                name=f"scb{it}")
                acc = evp.tile([128, SHARD_P], F16, tag="acc", bufs=2,
                               name=f"acc{it}")
                y = xc.tile([BATCH, SHARD_P], F32, tag="y", name=f"y{it}")

                qts = {}
                for j in range(NJ):
                    qts[j] = qtp.tile([128, SHARD_P], I16, tag="qt",
                                      name=f"qt{it}j{j}")

                # input DMAs on the SP ring; scb interleaved in 3 chunks
                # so late qt tiles are not starved behind the 2.9MB scb
                qtv = qt_d[:].rearrange("p (j o) -> p j o", j=NJ)
                scbv = scb_d[:].rearrange("p (j o) -> p j o", j=NJ)
                aeng = getattr(nc, aux_ring)
                for j in range(2):
                    nc.sync.dma_start(qts[j][:], qtv[:, j])
                aeng.dma_start(
                    mqc[:], mqc_d[:].rearrange("p (j c w) -> p j c w", j=NJ, c=8)
                )
                nc.sync.dma_start(scb[:, 0:3], scbv[:, 0:3])
                aeng.dma_start(aux[:], aux_d[:])
                for j in range(2, 4):
                    nc.sync.dma_start(qts[j][:], qtv[:, j])
                nc.sync.dma_start(scb[:, 3:6], scbv[:, 3:6])
                for j in range(4, 6):
                    nc.sync.dma_start(qts[j][:], qtv[:, j])
                nc.sync.dma_start(scb[:, 6:8], scbv[:, 6:8])
                for j in range(6, NJ):
                    nc.sync.dma_start(qts[j][:], qtv[:, j])

                # expand compact stationary into the masked mega-tile
                # (4 ops, one per 32-partition group block)
                mq16 = mqf[:].bitcast(I16).rearrange(
                    "p j pl xp i w -> p j (pl xp i) w"
                )
                for g in range(4):
                    sl = slice(32 * g, 32 * (g + 1))
                    nc.vector.tensor_copy(
                        mq16[sl, :, :, 16 * g:16 * (g + 1)], mqc[sl]
                    )

                z = aux[:, :SHARD_P]
                xs = aux[:, SHARD_P:]

                psum2 = ps2p.tile([BATCH, SHARD_P], F32, tag="ps2",
                                  name=f"ps2{it}")

                def emit_corr():
                    if "corr" in debug_skip:
                        return
                    for t in range(NT):
                        nc.tensor.matmul(
                            psum2[:, TO[t]:TO[t] + TW[t]],
                            xs[:],
                            z[:, TO[t]:TO[t] + TW[t]],
                            start=not bool(s2_per_j), stop=bool(s2_per_j),
                        )

                ev_k = 0
                add_k = 0
                pend_s2 = []

                def flush_s2(upto_j):
                    while pend_s2 and pend_s2[0][0] <= upto_j:
                        j2, t2, ssc2 = pend_s2.pop(0)
                        nc.tensor.matmul(
                            psum2[:, TO[t2]:TO[t2] + TW[t2]],
                            ind[:],
                            ssc2[:, :TW[t2]],
                            start=(j2 == s2_from_j), stop=False,
                        )

                for j in range(NJ):
                    flush_s2(j - s2_lag - 1)
                    v = qts[j]
                    A8 = plp.tile([128, SHARD_P], I16, tag="A8")
                    B8 = plp.tile([128, SHARD_P], I16, tag="B8")
                    if "unpack" not in debug_skip:
                        nc.vector.tensor_scalar(
                            A8[:], v[:], 0x0F0F, None,
                            mybir.AluOpType.bitwise_and,
                        )
                        nc.vector.tensor_scalar(
                            B8[:], v[:], 4, 0x0F0F,
                            mybir.AluOpType.logical_shift_right,
                            mybir.AluOpType.bitwise_and,
                        )
                    if j == 2 and not s2_per_j:
                        emit_corr()
                    planes = {
                        0: A8[:].bitcast(F8).rearrange(
                            "p (o two) -> p two o", two=2),
                        1: B8[:].bitcast(F8).rearrange(
                            "p (o two) -> p two o", two=2),
                    }
                    pss = [psp.tile([128, 512], F32, tag="ps1",
                                    name=f"ps1_{it}_{j}_{t}")
                           for t in range(NT)]
                    if "mm" not in debug_skip:
                        order = (
                            [(xp, pl, t) for xp in range(2) for pl in range(2)
                             for t in range(NT)]
                            if t_inner else
                            [(xp, pl, t) for t in range(NT) for xp in range(2)
                             for pl in range(2)]
                        )
                        for xp, pl, t in order:
                            w = TW[t]
                            k = 2 * xp + pl
                            if double_row:
                                nc.tensor.matmul(
                                    pss[t][:, :w],
                                    mqf[:, j, pl, xp],
                                    planes[pl][:, :, TO[t]:TO[t] + w],
                                    start=(k == 0), stop=(k == 3),
                                    perf_mode=mybir.MatmulPerfMode.DoubleRow,
                                )
                            else:
                                for i in range(2):
                                    nc.tensor.matmul(
                                        pss[t][:, :w],
                                        mqf[:, j, pl, xp, i],
                                        planes[pl][:, i, TO[t]:TO[t] + w],
                                        start=(k == 0 and i == 0),
                                        stop=(k == 3 and i == 1),
                                    )
                    # evict + scale + accumulate
                    if {"mm", "evict"} & set(debug_skip):
                        continue
                    for t in range(NT):
                        ps = pss[t]
                        w = TW[t]
                        scs = scb[:, j, TO[t]:TO[t] + w]
                        asl = acc[:, TO[t]:TO[t] + w]
                        if ev_k < dve_evict:
                            mulsrc = ps[:, :w]
                        else:
                            ev = evp.tile([128, 512], F16, tag="ev")
                            nc.scalar.activation(
                                ev[:, :w], ps[:, :w],
                                mybir.ActivationFunctionType.Copy,
                            )
                            mulsrc = ev[:, :w]
                        ev_k += 1
                        if s2_per_j and j >= s2_from_j:
                            ssc = evp.tile([128, 512], F16, tag="ssc")
                            nc.vector.tensor_tensor(
                                ssc[:, :w], mulsrc, scs, mybir.AluOpType.mult,
                            )
                            if s2_lag < 0:
                                nc.tensor.matmul(
                                    psum2[:, TO[t]:TO[t] + w],
                                    ind[:],
                                    ssc[:, :w],
                                    start=(j == s2_from_j), stop=False,
                                )
                            else:
                                pend_s2.append((j, t, ssc))
                        elif j == 0:
                            nc.vector.tensor_tensor(
                                asl, mulsrc, scs, mybir.AluOpType.mult,
                            )
                        else:
                            ssc = evp.tile([128, 512], F16, tag="ssc")
                            nc.vector.tensor_tensor(
                                ssc[:, :w], mulsrc, scs, mybir.AluOpType.mult,
                            )
                            eng = (nc.gpsimd if add_k < pool_adds
                                   else nc.vector)
                            eng.tensor_tensor(
                                asl, asl, ssc[:, :w], mybir.AluOpType.add,
                            )
                            add_k += 1

                # stage 2: fold group blocks + select batch
                if not ({"mm", "evict", "corr"} & set(debug_skip)):
                    flush_s2(NJ)
                    if s2_per_j:
                        if s2_from_j > 0:
                            for t in range(NT):
                                nc.tensor.matmul(
                                    psum2[:, TO[t]:TO[t] + TW[t]],
                                    ind[:],
                                    acc[:, TO[t]:TO[t] + TW[t]],
                                    start=False, stop=False,
                                )
                        emit_corr()
                    else:
                        for t in range(NT):
                            nc.tensor.matmul(
                                psum2[:, TO[t]:TO[t] + TW[t]],
                                ind[:],
                                acc[:, TO[t]:TO[t] + TW[t]],
                                start=False, stop=True,
                            )
                    nc.scalar.activation(
                        y[:], psum2[:], mybir.ActivationFunctionType.Copy
                    )
                else:
                    nc.vector.memset(y[:], 0.0)
                nc.gpsimd.dma_start(yt_d[:], y[:])

            if repeat == 1:
                body(0)
            else:
                U = unroll
                while repeat % U:
                    U -= 1
                with tc.For_i(
                    0, repeat // U, 1,
                    hint_engines=(
                        mybir.EngineType.PE,
                        mybir.EngineType.DVE,
                        mybir.EngineType.SP,
                        mybir.EngineType.Activation,
                        mybir.EngineType.Pool,
                    ),
                ):
                    for it in range(U):
                        body(it)

    nc.compile()
    return nc


def _f8(x):
    import ml_dtypes
    return x.astype(ml_dtypes.float8_e4m3fn)


def prep_inputs(x, qweight_packed, scales, zero_points, bias, perm,
                n_cores=N_CORES):
    """Host-side sharding/reshaping: byte permutation of weights + small-
    tensor math (x split, scale expansion, group sums)."""
    import ml_dtypes

    x = np.asarray(x, np.float32)
    qweight_packed = np.ascontiguousarray(np.asarray(qweight_packed, np.int32))
    scales = np.asarray(scales, np.float32)
    zero_points = np.asarray(zero_points, np.float32)
    bias = np.asarray(bias, np.float32)
    perm = np.asarray(perm, np.int64)
    shard = qweight_packed.shape[0] // n_cores

    # raw packed bytes: low byte of each little-endian int32
    qb = np.ascontiguousarray(
        qweight_packed.view(np.uint8).reshape(OUT_F, IN_F // 2, 4)[:, :, 0]
    )
    qb16_full = qb.view(np.int16)  # [OUT_F, 1024]; lane l = k 4l..4l+3

    x_perm = x[:, perm]                               # [B, IN_F]
    xh = _f8(x_perm)                                  # e4m3 high part
    xl = _f8(x_perm - xh.astype(np.float32))          # e4m3 residual
    xhat = xh.astype(np.float64) + xl.astype(np.float64)

    # compact stationary mqc[p, j, pl, xp, i, b] fp8 (viewed int16 pairs):
    # value = xpart[k, b] with k = 4*(128j+p) + (pl + 2i)
    k_idx = (4 * (128 * np.arange(NJ)[None, :, None, None]
                  + np.arange(128)[:, None, None, None])
             + np.arange(2)[None, None, :, None]
             + 2 * np.arange(2)[None, None, None, :])  # [p, j, pl, i]
    mqc = np.zeros((128, NJ, 2, 2, 2, BATCH), ml_dtypes.float8_e4m3fn)
    for xp, xpart in enumerate((xh, xl)):
        # xpart [B, IN_F] -> [p, j, pl, i, b]
        mqc[:, :, :, xp, :, :] = xpart.T[k_idx]       # [p,j,pl,i,B]
    mqc16 = np.ascontiguousarray(mqc).view(np.int16).reshape(128, NJ * 8 * 16)

    # group sums
    g_true = x_perm.astype(np.float64).T.reshape(N_GROUPS, GROUP, BATCH).sum(1)
    g_hat = xhat.T.reshape(N_GROUPS, GROUP, BATCH).sum(1)

    in_maps = []
    ind = np.zeros((128, BATCH), np.float16)
    ind[np.arange(128), np.arange(128) % BATCH] = 1.0
    for c in range(n_cores):
        sl = slice(c * shard, (c + 1) * shard)
        qt = np.zeros((128, NJ, SHARD_P), np.int16)
        qt[:, :, :shard] = (
            qb16_full[sl].T.reshape(NJ, 128, shard).transpose(1, 0, 2)
        )
        s_pad = np.zeros((SHARD_P, N_GROUPS), np.float32)
        s_pad[:shard] = scales[sl]
        s2 = (s_pad * 512.0).astype(np.float16)          # s * 2^9
        # scb[p, j, o] = s2[o, 4j + p//32]
        gsel = (4 * np.arange(NJ)[None, :]
                + (np.arange(128) // 32)[:, None])        # [p, j]
        scb = np.ascontiguousarray(
            s2.T[gsel]                                    # [p, j, SHARD_P]
        ).reshape(128, NJ * SHARD_P)
        aux = np.zeros((NZ, AUXW), np.float16)
        s_bf = s2.astype(np.float32) / 512.0              # bf16-rounded s
        aux[:N_GROUPS, :shard] = -8.0 * s_bf[:shard].T
        zp_pad = np.zeros((SHARD_P, N_GROUPS), np.float32)
        zp_pad[:shard] = zero_points[sl]
        aux[N_GROUPS:2 * N_GROUPS, :shard] = zp_pad[:shard].T
        aux[2 * N_GROUPS, :shard] = bias[sl]
        aux[:N_GROUPS, SHARD_P:] = g_hat
        aux[N_GROUPS:2 * N_GROUPS, SHARD_P:] = g_true
        aux[2 * N_GROUPS, SHARD_P:] = 1.0
        in_maps.append(
            {
                "qt": np.ascontiguousarray(qt).reshape(128, NJ * SHARD_P),
                "mqc": mqc16,
                "scb": scb,
                "aux": aux,
                "ind": ind,
            }
        )
    return in_maps


def assemble_output(results, n_cores=N_CORES, shard=SHARD):
    cols = []
    for c in range(n_cores):
        yt = np.asarray(results[c]["yT"], np.float32)     # [B, SHARD_P]
        cols.append(yt[:, :shard])
    return np.concatenate(cols, axis=1)


class _Runner:
    """Builds the program once; one jitted sharded executable reused across
    calls (same scheme as v2)."""

    def __init__(self, **build_kwargs):
        import jax
        from jax.sharding import Mesh, PartitionSpec, NamedSharding
        from jax.experimental.shard_map import shard_map
        from concourse import bass2jax

        self.jax = jax
        self.nc = build_nc(**build_kwargs)
        bass2jax.install_neuronx_cc_hook()
        nc = self.nc
        partition_name = (
            nc.partition_id_tensor.name if nc.partition_id_tensor else None
        )
        in_names, out_names, out_avals, zero_outs = [], [], [], []
        for alloc in nc.m.functions[0].allocations:
            if not isinstance(alloc, mybir.MemoryLocationSet):
                continue
            name = alloc.memorylocations[0].name
            if alloc.kind == "ExternalInput":
                if name != partition_name:
                    in_names.append(name)
            elif alloc.kind == "ExternalOutput":
                out_names.append(name)
                shape = tuple(alloc.tensor_shape)
                dtype = mybir.dt.np(alloc.dtype)
                out_avals.append(jax.core.ShapedArray(shape, dtype))
                zero_outs.append(np.zeros(shape, dtype))
        self.in_names, self.out_names = in_names, out_names
        self.out_avals, self.zero_outs = out_avals, zero_outs
        n_params, n_outs = len(in_names), len(out_avals)
        all_names = tuple(in_names + out_names)
        if partition_name is not None:
            all_names = all_names + (partition_name,)

        def _body(*args):
            operands = list(args)
            if partition_name is not None:
                operands.append(bass2jax.partition_id_tensor())
            outs = bass2jax._bass_exec_p.bind(
                *operands,
                out_avals=tuple(out_avals),
                in_names=all_names,
                out_names=tuple(out_names),
                lowering_input_output_aliases=(),
                sim_require_finite=True,
                sim_require_nnan=True,
                nc=nc,
            )
            return tuple(outs)

        devices = jax.devices()[:N_CORES]
        self.mesh = Mesh(np.asarray(devices), ("core",))
        in_specs = (PartitionSpec("core"),) * (n_params + n_outs)
        out_specs = (PartitionSpec("core"),) * n_outs
        self.sharded = jax.jit(
            shard_map(
                _body, mesh=self.mesh, in_specs=in_specs, out_specs=out_specs,
                check_rep=False,
            ),
            donate_argnums=tuple(range(n_params, n_params + n_outs)),
            keep_unused=True,
        )
        self.sharding = NamedSharding(self.mesh, PartitionSpec("core"))

    def put_inputs(self, in_maps):
        jax = self.jax
        arrs = [
            jax.device_put(
                np.concatenate(
                    [np.asarray(in_maps[c][n]) for c in range(N_CORES)], axis=0
                ),
                self.sharding,
            )
            for n in self.in_names
        ]
        for a in arrs:
            a.block_until_ready()
        return arrs

    def execute(self, dev_inputs):
        jax = self.jax
        zs = [
            jax.device_put(
                np.zeros((N_CORES * z.shape[0], *z.shape[1:]), z.dtype),
                self.sharding,
            )
            for z in self.zero_outs
        ]
        for z in zs:
            z.block_until_ready()
        outs = self.sharded(*dev_inputs, *zs)
        jax.block_until_ready(outs)
        return outs

    def run(self, in_maps):
        outs = self.execute(self.put_inputs(in_maps))
        res = []
        for c in range(N_CORES):
            d = {}
            for i, name in enumerate(self.out_names):
                d[name] = np.asarray(outs[i]).reshape(
                    N_CORES, *self.out_avals[i].shape
                )[c]
            res.append(d)
        return res


_RUNNER_CACHE = {}


def get_runner(**build_kwargs):
    key = tuple(sorted(build_kwargs.items()))
    if key not in _RUNNER_CACHE:
        _RUNNER_CACHE[key] = _Runner(**build_kwargs)
    return _RUNNER_CACHE[key]


def kernel(x, qweight_packed, scales, zero_points, bias, perm):
    runner = get_runner()
    in_maps = prep_inputs(x, qweight_packed, scales, zero_points, bias, perm)
    return assemble_output(runner.run(in_maps))


# revision 3
# speedup vs baseline: 1.0821x; 1.0821x over previous
"""GPTQ int4 linear kernel for Trainium2, 8-way sharded over out_features (v3).

y = x @ W_dq^T + bias; W_dq group-dequantized from int4 nibbles (two per
byte, only the low byte of each int32 of qweight_packed is meaningful).

v3 design (vs v2): flipped matmul orientation + fp8 subnormal decode.
  - Weights stream as the same host-pre-transposed int16 lane tiles
    qt[p, j, o] (byte pair = 4 nibbles for k = 4l..4l+3, l = 128j+p).
  - DVE decode, 2 passes per tile:  A8 = v & 0x0F0F, B8 = (v>>4) & 0x0F0F.
    Each int16 lane then holds TWO fp8e4m3 atoms whose bit patterns are
    raw nibbles: e4m3 pattern n (0..15) = n * 2^-9 EXACTLY (subnormals for
    n<8; verified exact on HW). A8 = (n@k=4l, n@k=4l+2), B8 = (4l+1, 4l+3).
  - Stage-1 matmuls run in fp8 DoubleRow perf mode (2 contraction rows per
    element pair, 0.5 cyc/col): stationary = block-diag-masked x as e4m3
    PAIRS, split-layout [128, 2, 128(g',b)] (walrus s3_lw_dual_fp8 rejects
    interleaved lhsT; interleaved RHS is fine, so the A8/B8 tiles are used
    directly via stride-2 fp8 views). x is split x = xh + xl (both e4m3,
    error feedback) -> 2 stationary sets; residual ~1.1e-3 relative.
  - PSUM layout [(g',b), o-third]: partitions = 4 group-blocks x 32 batch,
    free = 512 output cols (1 bank, bank-aligned). Accumulates A8/B8 x
    (xh, xl) = 4 matmuls per (j, third).
  - Evict+scale: ACT copies psum -> bf16 (some thirds go DVE-direct), DVE
    multiplies by scb (host-expanded s*2^9 in bf16, streamed per iter) and
    accumulates over j in bf16 (subnormal decode leaves no giant offsets,
    so bf16 is safe here).
  - Stage 2: one matmul per 512-col chunk with stationary = tiled identity
    IND[p, b] = (p%32 == b) contracts the (g',b) partitions -> psum2
    [32 b, o] f32; correction matmuls (zp vs true-x group sums, -8s vs
    (xh+xl) group sums, bias) accumulate into the same psum2 in fp16.
  - ACT evicts psum2 -> DMA out yT [32, 1408] f32.
Host does byte-permutation + small-tensor math only (scales/sums/x-split;
no weight-volume arithmetic).
"""

import sys

for _p in ("/opt/trn_rl_repo",):
    if _p not in sys.path:
        sys.path.insert(0, _p)

import numpy as np

import concourse.bacc as bacc
import concourse.bass as bass
import concourse.mybir as mybir
from concourse import tile

# Problem constants (hardcoded per contract)
OUT_F = 11008
IN_F = 4096
GROUP = 128
N_GROUPS = IN_F // GROUP  # 32
BATCH = 32
N_CORES = 8
SHARD = OUT_F // N_CORES      # 1376
SHARD_P = 1408                # padded to a multiple of 128
NJ = 8                        # int16 lane tiles (1024 lanes / 128)
NZ = 2 * N_GROUPS + 1         # correction rows: zp, -8s, bias
NT = 3                        # o-thirds (512, 512, 384)
TW = (512, 512, 384)
TO = (0, 512, 1024)

F32 = mybir.dt.float32
F16 = mybir.dt.float16
BF16 = mybir.dt.bfloat16
F8 = mybir.dt.float8e4
I16 = mybir.dt.int16

AUXW = SHARD_P + BATCH        # z cols | xs cols (fp16 rows 0..NZ)


def build_nc(repeat=1, unroll=16, dve_evict=0, pool_adds=8, double_row=True,
             ps_bufs=5, qt_bufs=16, pl_bufs=2, ev_bufs=6, scb_bufs=2,
             t_inner=True, s2_per_j=True, s2_from_j=0, s2_lag=-1,
             merge_evict=True,
             aux_ring="gpsimd",
             debug_skip=()):
    """Single-core program (SPMD across 8 cores, data differs).

    dve_evict: how many of the 24 (j,third) eviction slots skip the ACT
      copy and run the scale multiply directly on PSUM from DVE (1x mode)
      to balance ACT vs DVE.
    pool_adds: how many of the 21 j-accumulation adds run on the (idle)
      Pool engine instead of DVE.
    t_inner: order stage-1 matmuls (xp, pl) outer / t inner so consecutive
      matmuls share a stationary (Ldweights dedup probe).
    double_row: fp8 DoubleRow perf mode for stage-1 (fallback: plain fp8).
    """
    nc = bacc.Bacc("TRN2", target_bir_lowering=False, debug=False)

    qt_d = nc.dram_tensor("qt", [128, NJ * SHARD_P], I16, kind="ExternalInput")
    # compact stationary x: [p, j, pl, xp, i, b-pairs] int16 (fp8 pairs)
    mqc_d = nc.dram_tensor("mqc", [128, NJ * 8 * 16], I16, kind="ExternalInput")
    # expanded scales (s * 2^9, bf16): scb[p, j, o] = s2[o, 4j + p//32]
    scb_d = nc.dram_tensor("scb", [128, NJ * SHARD_P], F16, kind="ExternalInput")
    # aux fp16: rows 0..NZ: z (cols 0:1408) | xs (cols 1408:1440)
    aux_d = nc.dram_tensor("aux", [NZ, AUXW], F16, kind="ExternalInput")
    # stage-2 stationary: tiled identity, constant
    ind_d = nc.dram_tensor("ind", [128, BATCH], F16, kind="ExternalInput")
    yt_d = nc.dram_tensor("yT", [BATCH, SHARD_P], F32, kind="ExternalOutput")

    with tile.TileContext(nc) as tc:
        with (
            tc.tile_pool(name="xc", bufs=4) as xc,
            tc.tile_pool(name="qt", bufs=qt_bufs) as qtp,
            tc.tile_pool(name="pl", bufs=pl_bufs) as plp,
            tc.tile_pool(name="ev", bufs=ev_bufs) as evp,
            tc.tile_pool(name="scb", bufs=scb_bufs) as scbp,
            tc.tile_pool(name="ps", bufs=(2 if merge_evict else ps_bufs),
                         space="PSUM") as psp,
            tc.tile_pool(name="psb", bufs=1, space="PSUM") as psbp,
            tc.tile_pool(name="ps2", bufs=1, space="PSUM") as ps2p,
        ):
            # constants / fixed buffers (outside the hw loop)
            ind = xc.tile([128, BATCH], F16, tag="ind", bufs=1)
            nc.sync.dma_start(ind[:], ind_d[:])
            # masked stationary mega-tile [p, j, pl, xp, i, col] fp8,
            # zeroed once; diag blocks rewritten per body
            mqf = xc.tile([128, NJ, 2, 2, 2, 128], F8, tag="mqf", bufs=1)
            nc.vector.memset(mqf[:].bitcast(I16), 0)
            # ACT function table warm
            actwarm = xc.tile([128, 1], F32, tag="actwarm", bufs=1)
            nc.vector.memset(actwarm[:], 0.0)
            nc.scalar.activation(
                actwarm[:], actwarm[:], mybir.ActivationFunctionType.Copy
            )

            def body(it):
                mqc = xc.tile([128, NJ, 8, 16], I16, tag="mqc", name=f"mqc{it}")
                aux = xc.tile([NZ, AUXW], F16, tag="aux", name=f"aux{it}")
                scb = scbp.tile([128, NJ, SHARD_P], F16, tag="scb",
                                name=f"scb{it}")
                acc = evp.tile([128, SHARD_P], F16, tag="acc", bufs=2,
                               name=f"acc{it}")
                y = xc.tile([BATCH, SHARD_P], F32, tag="y", name=f"y{it}")

                qts = {}
                for j in range(NJ):
                    qts[j] = qtp.tile([128, SHARD_P], I16, tag="qt",
                                      name=f"qt{it}j{j}")

                # input DMAs on the SP ring; scb interleaved in 3 chunks
                # so late qt tiles are not starved behind the 2.9MB scb
                qtv = qt_d[:].rearrange("p (j o) -> p j o", j=NJ)
                scbv = scb_d[:].rearrange("p (j o) -> p j o", j=NJ)
                aeng = getattr(nc, aux_ring)
                for j in range(2):
                    nc.sync.dma_start(qts[j][:], qtv[:, j])
                aeng.dma_start(
                    mqc[:], mqc_d[:].rearrange("p (j c w) -> p j c w", j=NJ, c=8)
                )
                nc.sync.dma_start(scb[:, 0:3], scbv[:, 0:3])
                aeng.dma_start(aux[:], aux_d[:])
                for j in range(2, 4):
                    nc.sync.dma_start(qts[j][:], qtv[:, j])
                nc.sync.dma_start(scb[:, 3:6], scbv[:, 3:6])
                for j in range(4, 6):
                    nc.sync.dma_start(qts[j][:], qtv[:, j])
                nc.sync.dma_start(scb[:, 6:8], scbv[:, 6:8])
                for j in range(6, NJ):
                    nc.sync.dma_start(qts[j][:], qtv[:, j])

                # expand compact stationary into the masked mega-tile
                # (4 ops, one per 32-partition group block)
                mq16 = mqf[:].bitcast(I16).rearrange(
                    "p j pl xp i w -> p j (pl xp i) w"
                )
                for g in range(4):
                    sl = slice(32 * g, 32 * (g + 1))
                    nc.vector.tensor_copy(
                        mq16[sl, :, :, 16 * g:16 * (g + 1)], mqc[sl]
                    )

                z = aux[:, :SHARD_P]
                xs = aux[:, SHARD_P:]

                psum2 = ps2p.tile([BATCH, SHARD_P], F32, tag="ps2",
                                  name=f"ps2{it}")

                def emit_corr():
                    if "corr" in debug_skip:
                        return
                    for t in range(NT):
                        nc.tensor.matmul(
                            psum2[:, TO[t]:TO[t] + TW[t]],
                            xs[:],
                            z[:, TO[t]:TO[t] + TW[t]],
                            start=not bool(s2_per_j), stop=bool(s2_per_j),
                        )

                ev_k = 0
                add_k = 0
                pend_s2 = []

                def flush_s2(upto_j):
                    while pend_s2 and pend_s2[0][0] <= upto_j:
                        j2, t2, ssc2 = pend_s2.pop(0)
                        nc.tensor.matmul(
                            psum2[:, TO[t2]:TO[t2] + TW[t2]],
                            ind[:],
                            ssc2[:, :TW[t2]],
                            start=(j2 == s2_from_j), stop=False,
                        )

                for j in range(NJ):
                    flush_s2(j - s2_lag - 1)
                    v = qts[j]
                    A8 = plp.tile([128, SHARD_P], I16, tag="A8")
                    B8 = plp.tile([128, SHARD_P], I16, tag="B8")
                    if "unpack" not in debug_skip:
                        nc.vector.tensor_scalar(
                            A8[:], v[:], 0x0F0F, None,
                            mybir.AluOpType.bitwise_and,
                        )
                        nc.vector.tensor_scalar(
                            B8[:], v[:], 4, 0x0F0F,
                            mybir.AluOpType.logical_shift_right,
                            mybir.AluOpType.bitwise_and,
                        )
                    if j == 2 and not s2_per_j:
                        emit_corr()
                    planes = {
                        0: A8[:].bitcast(F8).rearrange(
                            "p (o two) -> p two o", two=2),
                        1: B8[:].bitcast(F8).rearrange(
                            "p (o two) -> p two o", two=2),
                    }
                    if merge_evict:
                        psa = psp.tile([128, 1024], F32, tag="psa",
                                       name=f"psa_{it}_{j}")
                        psb = psbp.tile([128, 384], F32, tag="psb",
                                        name=f"psb_{it}_{j}")
                        pss = [psa[:, 0:512], psa[:, 512:1024], psb[:, 0:384]]
                    else:
                        pss = [psp.tile([128, 512], F32, tag="ps1",
                                        name=f"ps1_{it}_{j}_{t}")[:]
                               for t in range(NT)]
                    if "mm" not in debug_skip:
                        order = (
                            [(xp, pl, t) for xp in range(2) for pl in range(2)
                             for t in range(NT)]
                            if t_inner else
                            [(xp, pl, t) for t in range(NT) for xp in range(2)
                             for pl in range(2)]
                        )
                        for xp, pl, t in order:
                            w = TW[t]
                            k = 2 * xp + pl
                            if double_row:
                                nc.tensor.matmul(
                                    pss[t][:, :w] if not merge_evict else pss[t][:, :w],
                                    mqf[:, j, pl, xp],
                                    planes[pl][:, :, TO[t]:TO[t] + w],
                                    start=(k == 0), stop=(k == 3),
                                    perf_mode=mybir.MatmulPerfMode.DoubleRow,
                                )
                            else:
                                for i in range(2):
                                    nc.tensor.matmul(
                                        pss[t][:, :w],
                                        mqf[:, j, pl, xp, i],
                                        planes[pl][:, i, TO[t]:TO[t] + w],
                                        start=(k == 0 and i == 0),
                                        stop=(k == 3 and i == 1),
                                    )
                    # evict + scale + accumulate
                    if {"mm", "evict"} & set(debug_skip):
                        continue
                    if merge_evict:
                        ev = evp.tile([128, SHARD_P], F16, tag="ev")
                        nc.scalar.activation(
                            ev[:, 0:1024], psa[:, 0:1024],
                            mybir.ActivationFunctionType.Copy,
                        )
                        nc.scalar.activation(
                            ev[:, 1024:SHARD_P], psb[:, 0:384],
                            mybir.ActivationFunctionType.Copy,
                        )
                        ssc = evp.tile([128, SHARD_P], F16, tag="ssc")
                        nc.vector.tensor_tensor(
                            ssc[:], ev[:], scb[:, j], mybir.AluOpType.mult,
                        )
                        for t in range(NT):
                            nc.tensor.matmul(
                                psum2[:, TO[t]:TO[t] + TW[t]],
                                ind[:],
                                ssc[:, TO[t]:TO[t] + TW[t]],
                                start=(j == s2_from_j), stop=False,
                            )
                        continue
                    for t in range(NT):
                        ps = pss[t]
                        w = TW[t]
                        scs = scb[:, j, TO[t]:TO[t] + w]
                        asl = acc[:, TO[t]:TO[t] + w]
                        if ev_k < dve_evict:
                            mulsrc = ps[:, :w]
                        else:
                            ev = evp.tile([128, 512], F16, tag="ev")
                            nc.scalar.activation(
                                ev[:, :w], ps[:, :w],
                                mybir.ActivationFunctionType.Copy,
                            )
                            mulsrc = ev[:, :w]
                        ev_k += 1
                        if s2_per_j and j >= s2_from_j:
                            ssc = evp.tile([128, 512], F16, tag="ssc")
                            nc.vector.tensor_tensor(
                                ssc[:, :w], mulsrc, scs, mybir.AluOpType.mult,
                            )
                            if s2_lag < 0:
                                nc.tensor.matmul(
                                    psum2[:, TO[t]:TO[t] + w],
                                    ind[:],
                                    ssc[:, :w],
                                    start=(j == s2_from_j), stop=False,
                                )
                            else:
                                pend_s2.append((j, t, ssc))
                        elif j == 0:
                            nc.vector.tensor_tensor(
                                asl, mulsrc, scs, mybir.AluOpType.mult,
                            )
                        else:
                            ssc = evp.tile([128, 512], F16, tag="ssc")
                            nc.vector.tensor_tensor(
                                ssc[:, :w], mulsrc, scs, mybir.AluOpType.mult,
                            )
                            eng = (nc.gpsimd if add_k < pool_adds
                                   else nc.vector)
                            eng.tensor_tensor(
                                asl, asl, ssc[:, :w], mybir.AluOpType.add,
                            )
                            add_k += 1

                # stage 2: fold group blocks + select batch
                if not ({"mm", "evict", "corr"} & set(debug_skip)):
                    flush_s2(NJ)
                    if s2_per_j:
                        if s2_from_j > 0:
                            for t in range(NT):
                                nc.tensor.matmul(
                                    psum2[:, TO[t]:TO[t] + TW[t]],
                                    ind[:],
                                    acc[:, TO[t]:TO[t] + TW[t]],
                                    start=False, stop=False,
                                )
                        emit_corr()
                    else:
                        for t in range(NT):
                            nc.tensor.matmul(
                                psum2[:, TO[t]:TO[t] + TW[t]],
                                ind[:],
                                acc[:, TO[t]:TO[t] + TW[t]],
                                start=False, stop=True,
                            )
                    nc.scalar.activation(
                        y[:], psum2[:], mybir.ActivationFunctionType.Copy
                    )
                else:
                    nc.vector.memset(y[:], 0.0)
                nc.gpsimd.dma_start(yt_d[:], y[:])

            if repeat == 1:
                body(0)
            else:
                U = unroll
                while repeat % U:
                    U -= 1
                with tc.For_i(
                    0, repeat // U, 1,
                    hint_engines=(
                        mybir.EngineType.PE,
                        mybir.EngineType.DVE,
                        mybir.EngineType.SP,
                        mybir.EngineType.Activation,
                        mybir.EngineType.Pool,
                    ),
                ):
                    for it in range(U):
                        body(it)

    nc.compile()
    return nc


def _f8(x):
    import ml_dtypes
    return x.astype(ml_dtypes.float8_e4m3fn)


def prep_inputs(x, qweight_packed, scales, zero_points, bias, perm,
                n_cores=N_CORES):
    """Host-side sharding/reshaping: byte permutation of weights + small-
    tensor math (x split, scale expansion, group sums)."""
    import ml_dtypes

    x = np.asarray(x, np.float32)
    qweight_packed = np.ascontiguousarray(np.asarray(qweight_packed, np.int32))
    scales = np.asarray(scales, np.float32)
    zero_points = np.asarray(zero_points, np.float32)
    bias = np.asarray(bias, np.float32)
    perm = np.asarray(perm, np.int64)
    shard = qweight_packed.shape[0] // n_cores

    # raw packed bytes: low byte of each little-endian int32
    qb = np.ascontiguousarray(
        qweight_packed.view(np.uint8).reshape(OUT_F, IN_F // 2, 4)[:, :, 0]
    )
    qb16_full = qb.view(np.int16)  # [OUT_F, 1024]; lane l = k 4l..4l+3

    x_perm = x[:, perm]                               # [B, IN_F]
    xh = _f8(x_perm)                                  # e4m3 high part
    xl = _f8(x_perm - xh.astype(np.float32))          # e4m3 residual
    xhat = xh.astype(np.float64) + xl.astype(np.float64)

    # compact stationary mqc[p, j, pl, xp, i, b] fp8 (viewed int16 pairs):
    # value = xpart[k, b] with k = 4*(128j+p) + (pl + 2i)
    k_idx = (4 * (128 * np.arange(NJ)[None, :, None, None]
                  + np.arange(128)[:, None, None, None])
             + np.arange(2)[None, None, :, None]
             + 2 * np.arange(2)[None, None, None, :])  # [p, j, pl, i]
    mqc = np.zeros((128, NJ, 2, 2, 2, BATCH), ml_dtypes.float8_e4m3fn)
    for xp, xpart in enumerate((xh, xl)):
        # xpart [B, IN_F] -> [p, j, pl, i, b]
        mqc[:, :, :, xp, :, :] = xpart.T[k_idx]       # [p,j,pl,i,B]
    mqc16 = np.ascontiguousarray(mqc).view(np.int16).reshape(128, NJ * 8 * 16)

    # group sums
    g_true = x_perm.astype(np.float64).T.reshape(N_GROUPS, GROUP, BATCH).sum(1)
    g_hat = xhat.T.reshape(N_GROUPS, GROUP, BATCH).sum(1)

    in_maps = []
    ind = np.zeros((128, BATCH), np.float16)
    ind[np.arange(128), np.arange(128) % BATCH] = 1.0
    for c in range(n_cores):
        sl = slice(c * shard, (c + 1) * shard)
        qt = np.zeros((128, NJ, SHARD_P), np.int16)
        qt[:, :, :shard] = (
            qb16_full[sl].T.reshape(NJ, 128, shard).transpose(1, 0, 2)
        )
        s_pad = np.zeros((SHARD_P, N_GROUPS), np.float32)
        s_pad[:shard] = scales[sl]
        s2 = (s_pad * 512.0).astype(np.float16)          # s * 2^9
        # scb[p, j, o] = s2[o, 4j + p//32]
        gsel = (4 * np.arange(NJ)[None, :]
                + (np.arange(128) // 32)[:, None])        # [p, j]
        scb = np.ascontiguousarray(
            s2.T[gsel]                                    # [p, j, SHARD_P]
        ).reshape(128, NJ * SHARD_P)
        aux = np.zeros((NZ, AUXW), np.float16)
        s_bf = s2.astype(np.float32) / 512.0              # bf16-rounded s
        aux[:N_GROUPS, :shard] = -8.0 * s_bf[:shard].T
        zp_pad = np.zeros((SHARD_P, N_GROUPS), np.float32)
        zp_pad[:shard] = zero_points[sl]
        aux[N_GROUPS:2 * N_GROUPS, :shard] = zp_pad[:shard].T
        aux[2 * N_GROUPS, :shard] = bias[sl]
        aux[:N_GROUPS, SHARD_P:] = g_hat
        aux[N_GROUPS:2 * N_GROUPS, SHARD_P:] = g_true
        aux[2 * N_GROUPS, SHARD_P:] = 1.0
        in_maps.append(
            {
                "qt": np.ascontiguousarray(qt).reshape(128, NJ * SHARD_P),
                "mqc": mqc16,
                "scb": scb,
                "aux": aux,
                "ind": ind,
            }
        )
    return in_maps


def assemble_output(results, n_cores=N_CORES, shard=SHARD):
    cols = []
    for c in range(n_cores):
        yt = np.asarray(results[c]["yT"], np.float32)     # [B, SHARD_P]
        cols.append(yt[:, :shard])
    return np.concatenate(cols, axis=1)


class _Runner:
    """Builds the program once; one jitted sharded executable reused across
    calls (same scheme as v2)."""

    def __init__(self, **build_kwargs):
        import jax
        from jax.sharding import Mesh, PartitionSpec, NamedSharding
        from jax.experimental.shard_map import shard_map
        from concourse import bass2jax

        self.jax = jax
        self.nc = build_nc(**build_kwargs)
        bass2jax.install_neuronx_cc_hook()
        nc = self.nc
        partition_name = (
            nc.partition_id_tensor.name if nc.partition_id_tensor else None
        )
        in_names, out_names, out_avals, zero_outs = [], [], [], []
        for alloc in nc.m.functions[0].allocations:
            if not isinstance(alloc, mybir.MemoryLocationSet):
                continue
            name = alloc.memorylocations[0].name
            if alloc.kind == "ExternalInput":
                if name != partition_name:
                    in_names.append(name)
            elif alloc.kind == "ExternalOutput":
                out_names.append(name)
                shape = tuple(alloc.tensor_shape)
                dtype = mybir.dt.np(alloc.dtype)
                out_avals.append(jax.core.ShapedArray(shape, dtype))
                zero_outs.append(np.zeros(shape, dtype))
        self.in_names, self.out_names = in_names, out_names
        self.out_avals, self.zero_outs = out_avals, zero_outs
        n_params, n_outs = len(in_names), len(out_avals)
        all_names = tuple(in_names + out_names)
        if partition_name is not None:
            all_names = all_names + (partition_name,)

        def _body(*args):
            operands = list(args)
            if partition_name is not None:
                operands.append(bass2jax.partition_id_tensor())
            outs = bass2jax._bass_exec_p.bind(
                *operands,
                out_avals=tuple(out_avals),
                in_names=all_names,
                out_names=tuple(out_names),
                lowering_input_output_aliases=(),
                sim_require_finite=True,
                sim_require_nnan=True,
                nc=nc,
            )
            return tuple(outs)

        devices = jax.devices()[:N_CORES]
        self.mesh = Mesh(np.asarray(devices), ("core",))
        in_specs = (PartitionSpec("core"),) * (n_params + n_outs)
        out_specs = (PartitionSpec("core"),) * n_outs
        self.sharded = jax.jit(
            shard_map(
                _body, mesh=self.mesh, in_specs=in_specs, out_specs=out_specs,
                check_rep=False,
            ),
            donate_argnums=tuple(range(n_params, n_params + n_outs)),
            keep_unused=True,
        )
        self.sharding = NamedSharding(self.mesh, PartitionSpec("core"))

    def put_inputs(self, in_maps):
        jax = self.jax
        arrs = [
            jax.device_put(
                np.concatenate(
                    [np.asarray(in_maps[c][n]) for c in range(N_CORES)], axis=0
                ),
                self.sharding,
            )
            for n in self.in_names
        ]
        for a in arrs:
            a.block_until_ready()
        return arrs

    def execute(self, dev_inputs):
        jax = self.jax
        zs = [
            jax.device_put(
                np.zeros((N_CORES * z.shape[0], *z.shape[1:]), z.dtype),
                self.sharding,
            )
            for z in self.zero_outs
        ]
        for z in zs:
            z.block_until_ready()
        outs = self.sharded(*dev_inputs, *zs)
        jax.block_until_ready(outs)
        return outs

    def run(self, in_maps):
        outs = self.execute(self.put_inputs(in_maps))
        res = []
        for c in range(N_CORES):
            d = {}
            for i, name in enumerate(self.out_names):
                d[name] = np.asarray(outs[i]).reshape(
                    N_CORES, *self.out_avals[i].shape
                )[c]
            res.append(d)
        return res


_RUNNER_CACHE = {}


def get_runner(**build_kwargs):
    key = tuple(sorted(build_kwargs.items()))
    if key not in _RUNNER_CACHE:
        _RUNNER_CACHE[key] = _Runner(**build_kwargs)
    return _RUNNER_CACHE[key]


def kernel(x, qweight_packed, scales, zero_points, bias, perm):
    runner = get_runner()
    in_maps = prep_inputs(x, qweight_packed, scales, zero_points, bias, perm)
    return assemble_output(runner.run(in_maps))
